# revision 1
# baseline (speedup 1.0000x reference)
"""Trainium2 Bass kernel for CRF score expansion.

Computes crf_scores[b, l, i, j] = emission[b, l, j] + transition[i, j]
for emission [32, 512, 64] f32 and transition [64, 64] f32, output
[32, 512, 64, 64] f32 (256 MB).

Sharding: data-parallel over the batch axis — 8 NeuronCores, 4 batches
(2048 (b,l) rows) per core; transition is replicated. No collectives.

Per-core kernel: the flattened transition (4096 f32) is broadcast once
to all 128 SBUF partitions (two 1 MB stride-0 DMAs, one per HWDGE
ring). Each of 16 steps loads 128 emission rows [128, 64], does a
single DVE tensor_add with the emission operand broadcast along the i
axis via a stride-0 access pattern, and DMAs the 2 MB result tile back
to DRAM. The kernel is HBM/SDMA-bound (32.5 MB/core through 16 SDMA
engines at ~27 GB/s each ≈ 90 us window); the DVE add stream (~70 us)
hides under the output DMA.
"""

import os
from contextlib import ExitStack

import numpy as np

B, L, T = 32, 512, 64
N_CORES = 8
B_PER = B // N_CORES          # 4 batches per core
R = B_PER * L                 # 2048 rows per core
P = 128                       # SBUF partitions per tile
N_TILES = R // P              # 16
TT = T * T                    # 4096

_cache = {}

# Set by each kernel() call when tracing is enabled (BASS_KERNEL_TRACE=1):
# the BassKernelResults from run_bass_kernel_spmd, for harnesses that want
# exec_time_ns / trace paths.
last_results = None


def _patch_sem_clear():
    """Replace the raw-ISA EVENT_SEMAPHORE_RANGE_CLEAR (opcode 176) with
    per-sem EventSemaphore writes.

    The walrus build in this container rejects the RANGE_CLEAR encoding
    ("ISA wrong length" in visitInstISA); plain InstEventSemaphore with a
    sem-wr-imm update is lowered by walrus itself and is equivalent for
    the small ranges Tile resets.
    """
    import concourse.bass as bass
    import concourse.mybir as mybir

    if getattr(bass.BassGpSimd, "_sem_clear_patched", False):
        return

    def sem_clear(self, sem):
        nums = list(sem) if isinstance(sem, range) else [sem.num]
        last = None
        for n in nums:
            upd = mybir.SyncUpdate(
                sync_type="semaphore",
                id=n,
                update_mode="sem-wr-imm",
                update_value=0,
                ant_name=f"sem_{n}",
            )
            ins = mybir.InstEventSemaphore(
                name=self.bass.get_next_instruction_name(),
                ins=[],
                outs=[],
                sync_info=mybir.SyncInfo(on_wait=[], on_update=[upd]),
            )
            last = self.add_instruction(ins)
        return last

    for cls in (
        bass.BassGpSimd,
        bass.BassVectorEngine,
        bass.BassScalarEngine,
        bass.BassTensorEngine,
    ):
        cls.sem_clear = sem_clear
    bass.BassGpSimd._sem_clear_patched = True


def _build_bass():
    import concourse.bass as bass
    import concourse.mybir as mybir
    import concourse.tile as tile
    from concourse import bacc

    _patch_sem_clear()

    f32 = mybir.dt.float32
    nc = bacc.Bacc("TRN2", target_bir_lowering=False, debug=False)

    em = nc.dram_tensor("emission", [R, T], f32, kind="ExternalInput")
    tr = nc.dram_tensor("transition", [T, T], f32, kind="ExternalInput")
    out = nc.dram_tensor("out", [R, TT], f32, kind="ExternalOutput")

    with ExitStack() as ctx:
        tc = ctx.enter_context(tile.TileContext(nc))
        const_pool = ctx.enter_context(tc.tile_pool(name="const", bufs=1))
        in_pool = ctx.enter_context(tc.tile_pool(name="in", bufs=4))
        out_pool = ctx.enter_context(tc.tile_pool(name="out", bufs=4))

        # Broadcast the flattened transition to all 128 partitions with
        # stride-0 DRAM-side APs (each partition re-reads the same row).
        # Split across both HWDGE rings so the two 1 MB halves fly in
        # parallel — this gates the first add.
        trb = const_pool.tile([P, TT], f32)
        tr_flat = tr[:].rearrange("a b -> (a b)").unsqueeze(0)
        H = TT // 2
        nc.sync.dma_start(trb[:, :H], tr_flat[:, :H].broadcast_to([P, H]))
        nc.scalar.dma_start(trb[:, H:], tr_flat[:, H:].broadcast_to([P, H]))

        for t in range(N_TILES):
            em_t = in_pool.tile([P, T], f32)
            # input DMAs ride the ACT HWDGE ring, outputs the SP ring, so
            # small loads never queue behind 2 MB stores.
            nc.scalar.dma_start(em_t[:], em[bass.ts(t, P), :])

            o_t = out_pool.tile([P, TT], f32)
            nc.vector.tensor_add(
                o_t[:].rearrange("p (i j) -> p i j", i=T),
                trb[:].rearrange("p (i j) -> p i j", i=T),
                em_t[:].unsqueeze(1).broadcast_to([P, T, T]),
            )
            nc.sync.dma_start(out[bass.ts(t, P), :], o_t[:])

    nc.compile()
    return nc


def _get_nc():
    if "nc" not in _cache:
        _cache["nc"] = _build_bass()
    return _cache["nc"]


def kernel(emission: np.ndarray, transition: np.ndarray) -> np.ndarray:
    global last_results
    from concourse.bass_utils import run_bass_kernel_spmd

    nc = _get_nc()

    em = np.ascontiguousarray(emission, dtype=np.float32).reshape(N_CORES, R, T)
    tr = np.ascontiguousarray(transition, dtype=np.float32)
    in_maps = [{"emission": em[i], "transition": tr} for i in range(N_CORES)]

    trace = bool(os.environ.get("BASS_KERNEL_TRACE"))
    res = run_bass_kernel_spmd(
        nc, in_maps, core_ids=list(range(N_CORES)), trace=trace
    )
    if trace:
        last_results = res

    full = np.stack([res.results[i]["out"] for i in range(N_CORES)])
    return full.reshape(B, L, T, T)

